# revision 21
# baseline (speedup 1.0000x reference)
"""Trainium2 Bass kernel for nn_BiLSTM_CRF_18098992185950 (8 NeuronCores).

Single-launch design (vs the 2-launch baseline):

  conv(2ch,k3,p1) + Linear(D->1) collapse into fixed 256-d projection vectors
  g_e0/g_e1/g_t0/g_t1 (see _gvec). Scores are dots of embedding rows with
  those vectors. The CRF forward DP in normal space is a matrix-product chain
  with the emit factor attached to the CURRENT index:
      Z = exp(emit_0)^T (prod_{t=0}^{1022} M_t) 1,
      M_t[j,k] = exp(sig(u_t[j] + v_{t+1}[k] + ct) + emit_{t+1}[k] - log s)
  Attaching emit to k (the free dim of each leaf) keeps every per-(t,k)
  quantity in row-major layout on device - no layout transposes for emit.

  Each core owns 128 leaves (t-shard). It indirect-DMA-gathers only its own
  129x64 candidate rows from the bf16 table (4.2 MB/core instead of
  streaming the full 102 MB f32 table), PE-transposes them (bf16, 1 cyc/row),
  projects against [g_t0, 0, g_t1, g_e1] to get u/ones/v/b rows, builds the
  64x64 leaves (PE outer-sum + ACT tanh/exp - sigmoid is computed as
  0.5*tanh(x/2)+0.5 so tanh+exp share ONE activation table set), and runs 32
  subchains of 4 leaves as batched 64x64 matmuls. The host combines the
  8*32 subchain matrices in f64 (pure glue, as in the baseline).
"""

import numpy as np

T = 1024
K = 64
D = 256
V = 100000
NCORES = 8
NFR = 129            # frames staged per core (128 + 1 overlap)
NBLK = 65            # 128-row gather tiles: 65*128 = 8320 >= 129*64
NROW = NBLK * 128    # 8320 (8256 real rows + 64 pad)
NG = 16              # leaf groups of 8 frames
NSUB = 32            # subchains per core
LSUB = 4             # leaves per subchain
NBATCH = 4           # chain batches of 8 subchains
GCH = 13             # gather chunk: blocks per indirect dma (5 chunks)

_PROG = {}


def _gvec(w3, l):
    g = np.zeros_like(l)
    g += w3[1] * l
    g[:-1] += w3[0] * l[1:]
    g[1:] += w3[2] * l[:-1]
    return g


def _mods():
    import concourse.bacc as bacc
    import concourse.mybir as mybir
    from concourse import tile, bass
    return bacc, mybir, tile, bass


def _build():
    if "p" in _PROG:
        return _PROG["p"]
    bacc, mybir, tile, bass = _mods()
    f32 = mybir.dt.float32
    bf16 = mybir.dt.bfloat16
    i32 = mybir.dt.int32
    AF = mybir.ActivationFunctionType
    OP = mybir.AluOpType

    nc = bacc.Bacc("TRN2", target_bir_lowering=False, debug=False,
                   enable_asserts=False, num_devices=NCORES)
    table = nc.dram_tensor("table", (V, D), bf16, kind="ExternalInput").ap()
    idx = nc.dram_tensor("idx", (128, NBLK), i32, kind="ExternalInput").ap()
    obs = nc.dram_tensor("obs", (256, D), bf16, kind="ExternalInput").ap()
    gmat = nc.dram_tensor("gmat", (D, 5), bf16, kind="ExternalInput").ap()
    ident = nc.dram_tensor("ident", (128, 128), bf16, kind="ExternalInput").ap()
    cvec = nc.dram_tensor("cvec", (1, 8), f32, kind="ExternalInput").ap()
    bias4 = nc.dram_tensor("bias4", (4, 1), f32, kind="ExternalInput").ap()
    addend = nc.dram_tensor("addend", (K, K), bf16, kind="ExternalInput").ap()
    qinit = nc.dram_tensor("qinit", (K, 512), bf16, kind="ExternalInput").ap()
    qout = nc.dram_tensor("qout", (NSUB * K, K), f32, kind="ExternalOutput").ap()
    em0out = nc.dram_tensor("em0out", (1, K), f32, kind="ExternalOutput").ap()
    dbg_buv = nc.dram_tensor("dbg_buv", (4, NROW), bf16, kind="ExternalOutput").ap()
    dbg_em = nc.dram_tensor("dbg_em", (NG, 512), bf16, kind="ExternalOutput").ap()
    emscr = nc.dram_tensor("emscr", (NG, 512), bf16, kind="Internal").ap()
    bscr = nc.dram_tensor("bscr", (1, NG * 512), bf16, kind="Internal").ap()
    ascr = nc.dram_tensor("ascr", (1, 256), f32, kind="Internal").ap()

    with tile.TileContext(nc) as tc:
        with (
            tc.tile_pool(name="persist", bufs=1) as pp,
            tc.tile_pool(name="ett", bufs=2) as ep,
            tc.tile_pool(name="grp", bufs=3) as gp,
            tc.tile_pool(name="qq", bufs=3) as qp,
            tc.tile_pool(name="ps_tr", bufs=2, space="PSUM") as ps_tr,
            tc.tile_pool(name="ps_pj", bufs=2, space="PSUM") as ps_pj,
            tc.tile_pool(name="ps_pl", bufs=2, space="PSUM") as ps_pl,
            tc.tile_pool(name="ps_pq", bufs=2, space="PSUM") as ps_pq,
        ):
            # ---- persistent inputs ----
            g_sb = pp.tile([128, 2, 5], bf16, tag="gmat")
            nc.sync.dma_start(g_sb[:], gmat.rearrange("(c p) g -> p c g", p=128))
            id_sb = pp.tile([128, 128], bf16, tag="ident")
            nc.sync.dma_start(id_sb[:], ident)
            idx_sb = pp.tile([128, NBLK], i32, tag="idx")
            nc.sync.dma_start(idx_sb[:], idx)
            obs_sb = pp.tile([128, 2, D], bf16, tag="obs")
            nc.sync.dma_start(obs_sb[:], obs.rearrange("(r p) d -> p r d", p=128))
            add_sb = pp.tile([K, K], bf16, tag="addend")
            nc.sync.dma_start(add_sb[:], addend)
            ct2_col = pp.tile([K, 1], f32, tag="ct2")
            nc.sync.dma_start(ct2_col[:], cvec[0:1, 0:1].to_broadcast((K, 1)))
            ce_col = pp.tile([4, 1], f32, tag="ce")
            nc.sync.dma_start(ce_col[:], cvec[0:1, 1:2].to_broadcast((4, 1)))
            c2_col = pp.tile([4, 1], f32, tag="c2")
            nc.sync.dma_start(c2_col[:], cvec[0:1, 2:3].to_broadcast((4, 1)))
            mask_col = pp.tile([K, 1], f32, tag="mask")
            nc.sync.dma_start(mask_col[:], cvec[0:1, 3:4].to_broadcast((K, 1)))
            ce2_col = pp.tile([1, 1], f32, tag="ce2")
            nc.sync.dma_start(ce2_col[:], cvec[0:1, 4:5])
            b4_col = pp.tile([4, 1], f32, tag="bias4")
            nc.sync.dma_start(b4_col[:], bias4)

            # ---- indirect gather of candidate rows, in chunks ----
            gt = pp.tile([128, NBLK, D], bf16, tag="gt")
            for gc in range(0, NBLK, GCH):
                hi = min(gc + GCH, NBLK)
                nc.gpsimd.indirect_dma_start(
                    out=gt[:, gc:hi, :],
                    out_offset=None,
                    in_=table,
                    in_offset=bass.IndirectOffsetOnAxis(
                        ap=idx_sb[:, gc:hi], axis=0),
                )

            # ---- obs -> a-row (1 x 256 frames) ----
            obt = ps_tr.tile([128, 512], bf16, tag="tr")
            for r in range(2):
                for ch in range(2):
                    nc.tensor.transpose(
                        out=obt[:, (r * 2 + ch) * 128:(r * 2 + ch + 1) * 128],
                        in_=obs_sb[:, r, ch * 128:(ch + 1) * 128],
                        identity=id_sb[:],
                    )
            obsT = pp.tile([128, 2, 2, 128], bf16, tag="obsT")  # [r, ch, x]
            nc.vector.tensor_copy(out=obsT[:].rearrange("p r c x -> p (r c x)"),
                                  in_=obt[:])
            arow_ps = ps_pj.tile([1, 256], f32, tag="pj")
            for ch in range(2):
                nc.tensor.matmul(
                    out=arow_ps[:],
                    lhsT=g_sb[:, ch, 4:5],
                    rhs=obsT[:, :, ch, :],
                    start=(ch == 0), stop=(ch == 1),
                )
            arow = pp.tile([1, 256], f32, tag="arow")
            nc.vector.tensor_copy(out=arow[:], in_=arow_ps[:])
            nc.sync.dma_start(out=ascr, in_=arow[:])
            a17s = []
            for c4 in range(4):
                a17c = pp.tile([4, 8], f32, tag=f"a17_{c4}")
                nc.sync.dma_start(
                    out=a17c[:],
                    in_=ascr.rearrange("o f -> (o f)")[
                        1 + 32 * c4: 1 + 32 * (c4 + 1)].rearrange(
                        "(p f) -> p f", p=4))
                a17s.append(a17c)

            # ---- transpose + project: buv rows [u, ones, v, b] ----
            buv = pp.tile([4, NROW], bf16, tag="buv")
            for pb in range(17):
                blo, bhi = pb * 4, min(pb * 4 + 4, NBLK)
                nb = bhi - blo
                w = nb * 128
                ett = ep.tile([128, 1024], bf16, tag="ett")  # [bi, ch, x]
                for q in range((nb + 1) // 2):
                    qb = min(2, nb - q * 2)
                    tp = ps_tr.tile([128, 512], bf16, tag="tr")
                    for bi in range(qb):
                        for ch in range(2):
                            nc.tensor.transpose(
                                out=tp[:, (bi * 2 + ch) * 128:(bi * 2 + ch + 1) * 128],
                                in_=gt[:, blo + q * 2 + bi, ch * 128:(ch + 1) * 128],
                                identity=id_sb[:],
                            )
                    w2 = qb * 256
                    if (pb + q) % 2 == 0:
                        nc.vector.tensor_copy(out=ett[:, q * 512: q * 512 + w2],
                                              in_=tp[:, :w2])
                    else:
                        nc.scalar.copy(out=ett[:, q * 512: q * 512 + w2],
                                       in_=tp[:, :w2])
                pj = ps_pj.tile([4, 512], f32, tag="pj")
                ett4 = ett[:].rearrange("p (b c x) -> p b c x", c=2, x=128)
                for ch in range(2):
                    nc.tensor.matmul(
                        out=pj[:, :w],
                        lhsT=g_sb[:, ch, 0:4],
                        rhs=ett4[:, :nb, ch, :],
                        start=(ch == 0), stop=(ch == 1),
                    )
                ev = (nc.vector, nc.scalar)[pb % 2]
                if ev is nc.vector:
                    nc.vector.tensor_scalar(
                        out=buv[:, pb * 512: pb * 512 + w], in0=pj[:, :w],
                        scalar1=b4_col[:], scalar2=None, op0=OP.add)
                else:
                    nc.scalar.activation(
                        buv[:, pb * 512: pb * 512 + w], pj[:, :w],
                        AF.Identity, bias=b4_col[:])

            # [ones; v] copy at base partition 0 (matmul rhs needs base 0)
            uv_r = pp.tile([2, NROW], bf16, tag="uv_r")
            for c5 in range(5):
                lo = c5 * 2048
                hi = min(lo + 2048, NROW)
                nc.sync.dma_start(out=uv_r[:, lo:hi], in_=buv[1:3, lo:hi])

            # ---- em' rows: em17[g, (i,k)] = tanh((a+b+ce)/2) + (2+2*mlogs) ----
            for c4 in range(4):
                p0, p1 = c4 * 4, c4 * 4 + 4
                nc.sync.dma_start(
                    out=bscr[:, 2048 * c4: 2048 * (c4 + 1)],
                    in_=buv[3:4, 64 + 2048 * c4: 64 + 2048 * (c4 + 1)],
                )
                b17c = pp.tile([4, 512], bf16, tag=f"b17_{c4}")
                nc.sync.dma_start(
                    out=b17c[:],
                    in_=bscr.rearrange("o f -> (o f)")[
                        2048 * c4: 2048 * (c4 + 1)].rearrange("(p f) -> p f", p=4),
                )
                th2c = pp.tile([4, 512], bf16, tag=f"th2_{c4}")
                nc.vector.scalar_tensor_tensor(
                    out=th2c[:].rearrange("p (i k) -> p i k", k=K),
                    in0=b17c[:].rearrange("p (i k) -> p i k", k=K),
                    scalar=ce_col[:],
                    in1=a17s[c4][:].unsqueeze(2).to_broadcast((4, 8, K)),
                    op0=OP.add, op1=OP.add,
                )
                em17c = pp.tile([4, 512], bf16, tag=f"em17_{c4}")
                nc.scalar.activation(em17c[:], th2c[:], AF.Tanh, scale=0.5)
                nc.scalar.activation(em17c[:], em17c[:],
                                     AF.Identity, bias=c2_col[:])
                nc.sync.dma_start(out=emscr[p0:p1, :], in_=em17c[:])
                nc.sync.dma_start(out=dbg_em[p0:p1, :], in_=em17c[:])

            # ---- emit_0 (only core 0's output is used) ----
            b0row = pp.tile([1, K], bf16, tag="b0row")
            nc.sync.dma_start(out=b0row[:], in_=buv[3:4, 0:K])
            s01 = pp.tile([1, K], f32, tag="s01")
            nc.vector.tensor_scalar(out=s01[:], in0=b0row[:],
                                    scalar1=arow[0:1, 0:1], scalar2=None, op0=OP.add)
            th0 = pp.tile([1, K], f32, tag="th0")
            nc.scalar.activation(th0[:], s01[:], AF.Tanh, bias=ce2_col[:], scale=0.5)
            em0sb = pp.tile([1, K], f32, tag="em0")
            nc.vector.tensor_scalar(out=em0sb[:], in0=th0[:], scalar1=0.5,
                                    scalar2=0.5, op0=OP.mult, op1=OP.add)
            nc.sync.dma_start(out=em0out, in_=em0sb[:])

            # em' broadcast across the 64 leaf partitions (DVE rejects
            # partition-stride-0 APs, so bounce through DRAM)
            embig = pp.tile([K, NG * 512], bf16, tag="embig")
            emflat = emscr.rearrange("p f -> (p f)").unsqueeze(0)
            for c4 in range(4):
                nc.sync.dma_start(
                    out=embig[:, c4 * 2048:(c4 + 1) * 2048],
                    in_=emflat[:, c4 * 2048:(c4 + 1) * 2048].to_broadcast(
                        (K, 2048)),
                )

            # ---- leaves: exp(0.5*(th1 + em')) ----
            leafbuf = pp.tile([K, 128 * K], bf16, tag="leafbuf")
            for g in range(NG):
                pl = ps_pl.tile([K, 512], f32, tag="pl")
                for q in range(8):
                    i = g * 8 + q
                    nc.tensor.matmul(
                        out=pl[:, q * K:(q + 1) * K],
                        lhsT=buv[0:2, i * K:(i + 1) * K],
                        rhs=uv_r[:, (i + 1) * K:(i + 2) * K],
                        start=True, stop=True,
                    )
                th1 = gp.tile([K, 512], bf16, tag="th1")
                nc.scalar.activation(th1[:], pl[:], AF.Tanh,
                                     bias=ct2_col[:], scale=0.5)
                st2 = gp.tile([K, 512], bf16, tag="st2")
                nc.vector.scalar_tensor_tensor(
                    out=st2[:], in0=th1[:], scalar=0.0,
                    in1=embig[:, g * 512:(g + 1) * 512],
                    op0=OP.add, op1=OP.add,
                )
                nc.scalar.activation(leafbuf[:, g * 512:(g + 1) * 512], st2[:],
                                     AF.Exp, scale=0.5)

            # pad leaf 127 -> mask*leaf + addend (identity/s on the last core)
            last = leafbuf[:, 127 * K:128 * K]
            nc.vector.scalar_tensor_tensor(
                out=last, in0=last, scalar=mask_col[:], in1=add_sb[:],
                op0=OP.mult, op1=OP.add,
            )

            # ---- chain: 4 batches of 8 subchains, 4 rounds each ----
            qout_sb = pp.tile([K, NSUB * K], f32, tag="qout_sb")
            for b in range(NBATCH):
                qcur = qp.tile([K, 512], bf16, tag="q")
                nc.sync.dma_start(qcur[:], qinit)
                for r in range(LSUB):
                    pq = ps_pq.tile([K, 512], f32, tag="pq")
                    for s8 in range(8):
                        t = 4 * (8 * b + s8) + r
                        nc.tensor.matmul(
                            out=pq[:, s8 * K:(s8 + 1) * K],
                            lhsT=leafbuf[:, t * K:(t + 1) * K],
                            rhs=qcur[:, s8 * K:(s8 + 1) * K],
                            start=True, stop=True,
                        )
                    if r < LSUB - 1:
                        qnext = qp.tile([K, 512], bf16, tag="q")
                        ev = (nc.vector, nc.scalar)[r % 2]
                        if ev is nc.vector:
                            ev.tensor_copy(out=qnext[:], in_=pq[:])
                        else:
                            ev.copy(out=qnext[:], in_=pq[:])
                        qcur = qnext
                    else:
                        ev = (nc.vector, nc.scalar)[b % 2]
                        if ev is nc.vector:
                            ev.tensor_copy(out=qout_sb[:, b * 512:(b + 1) * 512],
                                           in_=pq[:])
                        else:
                            ev.copy(out=qout_sb[:, b * 512:(b + 1) * 512],
                                    in_=pq[:])
            nc.sync.dma_start(
                out=qout.rearrange("(s j) k -> j s k", s=NSUB),
                in_=qout_sb[:].rearrange("p (s k) -> p s k", k=K),
            )
            nc.sync.dma_start(out=dbg_buv, in_=buv[:])
    nc.compile()
    _PROG["p"] = nc
    return nc


def _host_consts(inputs):
    E = np.asarray(inputs["word_embeds"], dtype=np.float32)
    ids = np.asarray(inputs["candidate_ids"]).astype(np.int64)
    obs = np.ascontiguousarray(np.asarray(inputs["observed_feats"], dtype=np.float32))

    lw_e = np.asarray(inputs["emit_lin_w"], dtype=np.float64)[0]
    lw_t = np.asarray(inputs["trans_lin_w"], dtype=np.float64)[0]
    cw_e = np.asarray(inputs["emit_conv_w"], dtype=np.float64)
    cw_t = np.asarray(inputs["trans_conv_w"], dtype=np.float64)
    g_e0 = _gvec(cw_e[0, 0], lw_e)
    g_e1 = _gvec(cw_e[0, 1], lw_e)
    g_t0 = _gvec(cw_t[0, 0], lw_t)
    g_t1 = _gvec(cw_t[0, 1], lw_t)
    ce = float(np.asarray(inputs["emit_conv_b"], np.float64)[0] * lw_e.sum()
               + np.asarray(inputs["emit_lin_b"], np.float64)[0])
    ct = float(np.asarray(inputs["trans_conv_b"], np.float64)[0] * lw_t.sum()
               + np.asarray(inputs["trans_lin_b"], np.float64)[0])

    samp = E[ids[:8].ravel()].astype(np.float64)
    sig = 1.0 / (1.0 + np.exp(-((samp @ g_t0).mean() + (samp @ g_t1).mean() + ct)))
    a8 = obs[:8].astype(np.float64) @ g_e0
    em = 1.0 / (1.0 + np.exp(-(a8.mean() + (samp @ g_e1).mean() + ce)))
    s = float(64.0 * np.exp(sig + em))
    gmat = np.stack([g_t0, np.zeros(D), g_t1, g_e1, g_e0], axis=1)
    return E, ids, obs, gmat, ce, ct, s


def _run_launch(inputs, run_kw=None):
    from concourse.bass_utils import run_bass_kernel_spmd
    import ml_dtypes

    bf16 = ml_dtypes.bfloat16
    run_kw = run_kw or {}
    E, ids, obs, gmat, ce, ct, s = _host_consts(inputs)
    mlogs = -np.log(s)

    Eb = np.ascontiguousarray(E.astype(bf16))
    gmb = np.ascontiguousarray(gmat.astype(np.float32).astype(bf16))
    identb = np.eye(128, dtype=np.float32).astype(bf16)
    qinitb = np.ascontiguousarray(
        np.tile(np.eye(K, dtype=np.float32), (1, 8)).astype(bf16))
    eye_s = (np.eye(K, dtype=np.float64) / s).astype(np.float32).astype(bf16)
    zer = np.zeros((K, K), dtype=bf16)
    bias4 = np.array([[0.0], [1.0], [0.0], [0.0]], dtype=np.float32)

    ids_pad = np.zeros((T + 1, K), dtype=np.int64)
    ids_pad[:T] = ids

    prog = _build()
    in_maps = []
    for c in range(NCORES):
        fr0 = 128 * c
        rid = ids_pad[fr0:fr0 + NFR].ravel()
        rid_pad = np.zeros(NROW, dtype=np.int64)
        rid_pad[:rid.size] = rid
        idx_c = np.ascontiguousarray(
            rid_pad.reshape(NBLK, 128).T.astype(np.int32))
        obs_c = np.zeros((256, D), dtype=np.float32)
        n = min(NFR, T - fr0)
        obs_c[:n] = obs[fr0:fr0 + n]
        cv = np.zeros((1, 8), dtype=np.float32)
        cv[0, 0] = ct / 2.0
        cv[0, 1] = ce
        cv[0, 2] = 2.0 + 2.0 * mlogs
        cv[0, 3] = 0.0 if c == NCORES - 1 else 1.0
        cv[0, 4] = ce / 2.0
        in_maps.append({
            "table": Eb,
            "idx": idx_c,
            "obs": np.ascontiguousarray(obs_c.astype(bf16)),
            "gmat": gmb,
            "ident": identb,
            "cvec": cv,
            "bias4": bias4,
            "addend": eye_s if c == NCORES - 1 else zer,
            "qinit": qinitb,
        })
    res = run_bass_kernel_spmd(prog, in_maps, core_ids=list(range(NCORES)),
                               **run_kw)

    # ---- host combine in f64 ----
    P = np.eye(K, dtype=np.float64)
    acc = 0.0
    for c in range(NCORES):
        qo = res.results[c]["qout"].astype(np.float64)
        for sc in range(NSUB):
            P = P @ qo[sc * K:(sc + 1) * K, :].T
            m = np.abs(P).max()
            P /= m
            acc += np.log(m)
    em0 = res.results[0]["em0out"][0].astype(np.float64)
    z = np.exp(em0) @ P @ np.ones(K)
    ans = np.log(z) + acc + NSUB * LSUB * NCORES * np.log(np.float64(s))
    return np.array([ans], dtype=np.float32), res


def kernel(**inputs):
    ans, _ = _run_launch(inputs)
    return ans


def profiled_run(inputs):
    """Run the launch with NTFF tracing; return exec ns (or None)."""
    import sys as _sys
    import types as _types
    try:
        if "antenv.axon_hooks" not in _sys.modules:
            from trn_agent_boot.trn_boot import _ntff_profile_via_ctypes
            hook = _ntff_profile_via_ctypes("/opt/axon/libaxon_pjrt.so")
            mod = _types.ModuleType("antenv.axon_hooks")
            mod.get_axon_ntff_profile_hook = lambda: hook
            mod.set_axon_ntff_profile_hook = lambda h: None
            _sys.modules["antenv.axon_hooks"] = mod
            import antenv
            antenv.axon_hooks = mod
    except Exception as e:
        print(f"profile shim unavailable: {e}")
        return None
    ans, res = _run_launch(inputs, run_kw={"trace": True, "trace_cores": [0]})
    print("profiled answer:", ans)
    tr = res.instructions_and_trace
    print(f"P: exec_time_ns={res.exec_time_ns}"
          + (f" trace={tr[1]}" if tr else ""))
    return res.exec_time_ns


# revision 22
# speedup vs baseline: 4.3874x; 4.3874x over previous
"""Trainium2 Bass kernel for nn_BiLSTM_CRF_18098992185950 (8 NeuronCores).

Two launches, bf16 datapath (tolerance is 2e-2; bf16 lands ~3e-4):

  conv(2ch,k3,p1) + Linear(D->1) collapse into fixed 256-d projection
  vectors g_e0/g_e1/g_t0/g_t1 (see _gvec), so all scores are dots of
  embedding rows with 4 fixed vectors. The CRF forward DP in normal space
  is a matrix chain with emit attached to the CURRENT index:
      Z = exp(emit_0)^T (prod_{t=0}^{1022} M_t) 1,
      M_t[j,k] = exp(sig(u_t[j] + v_{t+1}[k] + ct) + emit_{t+1}[k] - log s)
  Emit on k (the free dim of each leaf) keeps every per-(t,k) quantity
  row-major on device - no layout transposes anywhere.

  L1 (memory-bound): the host stages the embedding table TRANSPOSED
  (256, V) in bf16; each core streams its V/8 shard sequentially (6.4 MB)
  and computes proj = [g_t0|g_t1|g_e1]^T @ tableT with plain matmuls -
  no PE transposes (d is already on partitions). obs is staged transposed
  too, giving the a-row the same way. (An on-device indirect row gather
  was tried instead: random 512B DMA descriptors run latency-bound at
  ~1.4 GB/s/engine, 30x slower than this sequential stream.)

  Host glue: gathers proj[:, candidate_ids] (~50 KB/core) and restages -
  pure indexing, like the baseline.

  L2 (compute): per core, 128 leaves built with K=2 outer-sum matmuls +
  ACT tanh (sigmoid via 0.5*tanh(x/2)+0.5 so tanh and exp share ONE
  activation table - no 1.3us table reloads) + DVE add of the emit row
  (broadcast via a DRAM bounce) + ACT exp into bf16; then 32 subchains
  of 4 leaves advance as batched 64x64 bf16 matmuls. The host combines
  the 8*32 subchain matrices in f64 (as the baseline did).
"""

import numpy as np

T = 1024
K = 64
D = 256
V = 100000
NCORES = 8
VSH = 12544          # V-shard columns per core (8*12544 >= V)
NFR = 129            # frames per core (128 + 1 overlap)
NROW = 8320          # staged (t,k) rows: 129*64 = 8256, padded
NG = 16              # leaf groups of 8 frames
NSUB = 32            # subchains per core
LSUB = 4             # leaves per subchain
NBATCH = 4           # chain batches of 8 subchains
LTW = 2048           # L1 stream tile width (columns)

_PROG = {}


def _gvec(w3, l):
    g = np.zeros_like(l)
    g += w3[1] * l
    g[:-1] += w3[0] * l[1:]
    g[1:] += w3[2] * l[:-1]
    return g


def _mods():
    import concourse.bacc as bacc
    import concourse.mybir as mybir
    from concourse import tile, bass
    return bacc, mybir, tile, bass


def _build_l1():
    if "l1" in _PROG:
        return _PROG["l1"]
    bacc, mybir, tile, bass = _mods()
    f32 = mybir.dt.float32
    bf16 = mybir.dt.bfloat16

    nc = bacc.Bacc("TRN2", target_bir_lowering=False, debug=False,
                   enable_asserts=False, num_devices=NCORES)
    ttc = nc.dram_tensor("ttc", (D, VSH), bf16, kind="ExternalInput").ap()
    obsT = nc.dram_tensor("obsT", (D, 256), bf16, kind="ExternalInput").ap()
    gmat = nc.dram_tensor("gmat", (D, 5), bf16, kind="ExternalInput").ap()
    projout = nc.dram_tensor("projout", (3, VSH), bf16,
                             kind="ExternalOutput").ap()
    arowout = nc.dram_tensor("arowout", (1, 256), f32,
                             kind="ExternalOutput").ap()

    ttr = ttc.rearrange("(c p) v -> p c v", p=128)
    with tile.TileContext(nc) as tc:
        with (
            tc.tile_pool(name="persist", bufs=1) as pp,
            tc.tile_pool(name="load", bufs=3) as lp,
            tc.tile_pool(name="out", bufs=3) as op,
            tc.tile_pool(name="ps_pj", bufs=4, space="PSUM") as ps_pj,
        ):
            g_sb = pp.tile([128, 2, 5], bf16, tag="gmat")
            nc.sync.dma_start(g_sb[:], gmat.rearrange("(c p) g -> p c g", p=128))

            # a-row from transposed obs: 2 matmuls, no transposes
            obsT_sb = pp.tile([128, 2, 256], bf16, tag="obsT")
            nc.sync.dma_start(obsT_sb[:],
                              obsT.rearrange("(c p) f -> p c f", p=128))
            arow_ps = ps_pj.tile([1, 256], f32, tag="ar")
            for ch in range(2):
                nc.tensor.matmul(
                    out=arow_ps[:], lhsT=g_sb[:, ch, 3:4],
                    rhs=obsT_sb[:, ch, :], start=(ch == 0), stop=(ch == 1),
                )
            arow = pp.tile([1, 256], f32, tag="arow")
            nc.vector.tensor_copy(out=arow[:], in_=arow_ps[:])
            nc.sync.dma_start(out=arowout, in_=arow[:])

            # stream the tableT shard; proj rows [u, v, b]
            for vt in range((VSH + LTW - 1) // LTW):
                lo = vt * LTW
                w = min(LTW, VSH - lo)
                tt = lp.tile([128, 2, LTW], bf16, tag="tt")
                nc.sync.dma_start(tt[:, :, :w], ttr[:, :, lo:lo + w])
                pr = op.tile([3, LTW], bf16, tag="pr")
                for j in range((w + 511) // 512):
                    w2 = min(512, w - j * 512)
                    pj = ps_pj.tile([3, 512], f32, tag="pj")
                    for ch in range(2):
                        nc.tensor.matmul(
                            out=pj[:, :w2],
                            lhsT=g_sb[:, ch, 0:3],
                            rhs=tt[:, ch, j * 512: j * 512 + w2],
                            start=(ch == 0), stop=(ch == 1),
                        )
                    if (vt + j) % 2 == 0:
                        nc.vector.tensor_copy(
                            out=pr[:, j * 512: j * 512 + w2], in_=pj[:, :w2])
                    else:
                        nc.scalar.copy(
                            out=pr[:, j * 512: j * 512 + w2], in_=pj[:, :w2])
                nc.sync.dma_start(out=projout[:, lo:lo + w], in_=pr[:, :w])
    nc.compile()
    _PROG["l1"] = nc
    return nc


def _build_l2():
    if "l2" in _PROG:
        return _PROG["l2"]
    bacc, mybir, tile, bass = _mods()
    f32 = mybir.dt.float32
    bf16 = mybir.dt.bfloat16
    AF = mybir.ActivationFunctionType
    OP = mybir.AluOpType

    nc = bacc.Bacc("TRN2", target_bir_lowering=False, debug=False,
                   enable_asserts=False, num_devices=NCORES)
    buv_in = nc.dram_tensor("buv", (4, NROW), bf16, kind="ExternalInput").ap()
    uvr_in = nc.dram_tensor("uvr", (2, NROW), bf16, kind="ExternalInput").ap()
    b17_in = nc.dram_tensor("b17", (NG, 512), bf16, kind="ExternalInput").ap()
    a17_in = nc.dram_tensor("a17", (NG, 8), f32, kind="ExternalInput").ap()
    b0_in = nc.dram_tensor("b0", (1, K), bf16, kind="ExternalInput").ap()
    cvec = nc.dram_tensor("cvec", (1, 8), f32, kind="ExternalInput").ap()
    addend = nc.dram_tensor("addend", (K, K), bf16, kind="ExternalInput").ap()
    qinit = nc.dram_tensor("qinit", (K, 512), bf16, kind="ExternalInput").ap()
    qout = nc.dram_tensor("qout", (NSUB * K, K), f32, kind="ExternalOutput").ap()
    em0out = nc.dram_tensor("em0out", (1, K), f32, kind="ExternalOutput").ap()
    dbg_em = nc.dram_tensor("dbg_em", (NG, 512), bf16, kind="ExternalOutput").ap()
    emscr = nc.dram_tensor("emscr", (NG, 512), bf16, kind="Internal").ap()

    with tile.TileContext(nc) as tc:
        with (
            tc.tile_pool(name="persist", bufs=1) as pp,
            tc.tile_pool(name="grp", bufs=3) as gp,
            tc.tile_pool(name="qq", bufs=3) as qp,
            tc.tile_pool(name="ps_pl", bufs=2, space="PSUM") as ps_pl,
            tc.tile_pool(name="ps_pq", bufs=2, space="PSUM") as ps_pq,
        ):
            buv = pp.tile([4, NROW], bf16, tag="buv")
            nc.sync.dma_start(buv[:], buv_in)
            uv_r = pp.tile([2, NROW], bf16, tag="uvr")
            nc.sync.dma_start(uv_r[:], uvr_in)
            b17 = pp.tile([NG, 512], bf16, tag="b17")
            nc.sync.dma_start(b17[:], b17_in)
            a17 = pp.tile([NG, 8], f32, tag="a17")
            nc.sync.dma_start(a17[:], a17_in)
            b0row = pp.tile([1, K], bf16, tag="b0")
            nc.sync.dma_start(b0row[:], b0_in)
            add_sb = pp.tile([K, K], bf16, tag="addend")
            nc.sync.dma_start(add_sb[:], addend)
            ct2_col = pp.tile([K, 1], f32, tag="ct2")
            nc.sync.dma_start(ct2_col[:], cvec[0:1, 0:1].to_broadcast((K, 1)))
            ce_col = pp.tile([NG, 1], f32, tag="ce")
            nc.sync.dma_start(ce_col[:], cvec[0:1, 1:2].to_broadcast((NG, 1)))
            c2_col = pp.tile([NG, 1], f32, tag="c2")
            nc.sync.dma_start(c2_col[:], cvec[0:1, 2:3].to_broadcast((NG, 1)))
            mask_col = pp.tile([K, 1], f32, tag="mask")
            nc.sync.dma_start(mask_col[:], cvec[0:1, 3:4].to_broadcast((K, 1)))
            ce2_col = pp.tile([1, 1], f32, tag="ce2")
            nc.sync.dma_start(ce2_col[:], cvec[0:1, 4:5])
            a0_col = pp.tile([1, 1], f32, tag="a0")
            nc.sync.dma_start(a0_col[:], cvec[0:1, 5:6])

            # ---- em' = tanh(0.5*(a+b+ce)) + (2+2*mlogs), frames 1..128 ----
            th2 = pp.tile([NG, 512], bf16, tag="th2")
            nc.vector.scalar_tensor_tensor(
                out=th2[:].rearrange("p (i k) -> p i k", k=K),
                in0=b17[:].rearrange("p (i k) -> p i k", k=K),
                scalar=ce_col[:],
                in1=a17[:].unsqueeze(2).to_broadcast((NG, 8, K)),
                op0=OP.add, op1=OP.add,
            )
            em17 = pp.tile([NG, 512], bf16, tag="em17")
            nc.scalar.activation(em17[:], th2[:], AF.Tanh, scale=0.5)
            nc.scalar.activation(em17[:], em17[:], AF.Identity, bias=c2_col[:])
            nc.sync.dma_start(out=emscr, in_=em17[:])
            nc.sync.dma_start(out=dbg_em, in_=em17[:])

            # broadcast em' across the 64 leaf partitions via DRAM bounce
            embig = pp.tile([K, NG * 512], bf16, tag="embig")
            emflat = emscr.rearrange("p f -> (p f)").unsqueeze(0)
            for c4 in range(4):
                nc.sync.dma_start(
                    out=embig[:, c4 * 2048:(c4 + 1) * 2048],
                    in_=emflat[:, c4 * 2048:(c4 + 1) * 2048].to_broadcast(
                        (K, 2048)),
                )

            # ---- emit_0 ----
            s01 = pp.tile([1, K], f32, tag="s01")
            nc.vector.tensor_scalar(out=s01[:], in0=b0row[:],
                                    scalar1=a0_col[:], scalar2=None, op0=OP.add)
            th0 = pp.tile([1, K], f32, tag="th0")
            nc.scalar.activation(th0[:], s01[:], AF.Tanh, bias=ce2_col[:],
                                 scale=0.5)
            em0sb = pp.tile([1, K], f32, tag="em0")
            nc.vector.tensor_scalar(out=em0sb[:], in0=th0[:], scalar1=0.5,
                                    scalar2=0.5, op0=OP.mult, op1=OP.add)
            nc.sync.dma_start(out=em0out, in_=em0sb[:])

            # ---- leaves: exp(0.5*(th1 + em')) ----
            leafbuf = pp.tile([K, 128 * K], bf16, tag="leafbuf")
            for g in range(NG):
                pl = ps_pl.tile([K, 512], f32, tag="pl")
                for q in range(8):
                    i = g * 8 + q
                    nc.tensor.matmul(
                        out=pl[:, q * K:(q + 1) * K],
                        lhsT=buv[0:2, i * K:(i + 1) * K],
                        rhs=uv_r[:, (i + 1) * K:(i + 2) * K],
                        start=True, stop=True,
                    )
                th1 = gp.tile([K, 512], bf16, tag="th1")
                nc.scalar.activation(th1[:], pl[:], AF.Tanh,
                                     bias=ct2_col[:], scale=0.5)
                st2 = gp.tile([K, 512], bf16, tag="st2")
                nc.vector.scalar_tensor_tensor(
                    out=st2[:], in0=th1[:], scalar=0.0,
                    in1=embig[:, g * 512:(g + 1) * 512],
                    op0=OP.add, op1=OP.add,
                )
                nc.scalar.activation(leafbuf[:, g * 512:(g + 1) * 512], st2[:],
                                     AF.Exp, scale=0.5)

            # pad leaf 127 -> mask*leaf + addend (identity/s on the last core)
            last = leafbuf[:, 127 * K:128 * K]
            nc.vector.scalar_tensor_tensor(
                out=last, in0=last, scalar=mask_col[:], in1=add_sb[:],
                op0=OP.mult, op1=OP.add,
            )

            # ---- chain: 4 batches of 8 subchains, 4 rounds each ----
            qout_sb = pp.tile([K, NSUB * K], f32, tag="qout_sb")
            for b in range(NBATCH):
                qcur = qp.tile([K, 512], bf16, tag="q")
                nc.sync.dma_start(qcur[:], qinit)
                for r in range(LSUB):
                    pq = ps_pq.tile([K, 512], f32, tag="pq")
                    for s8 in range(8):
                        t = 4 * (8 * b + s8) + r
                        nc.tensor.matmul(
                            out=pq[:, s8 * K:(s8 + 1) * K],
                            lhsT=leafbuf[:, t * K:(t + 1) * K],
                            rhs=qcur[:, s8 * K:(s8 + 1) * K],
                            start=True, stop=True,
                        )
                    if r < LSUB - 1:
                        qnext = qp.tile([K, 512], bf16, tag="q")
                        if r % 2 == 0:
                            nc.vector.tensor_copy(out=qnext[:], in_=pq[:])
                        else:
                            nc.scalar.copy(out=qnext[:], in_=pq[:])
                        qcur = qnext
                    elif b % 2 == 0:
                        nc.vector.tensor_copy(
                            out=qout_sb[:, b * 512:(b + 1) * 512], in_=pq[:])
                    else:
                        nc.scalar.copy(
                            out=qout_sb[:, b * 512:(b + 1) * 512], in_=pq[:])
            nc.sync.dma_start(
                out=qout.rearrange("(s j) k -> j s k", s=NSUB),
                in_=qout_sb[:].rearrange("p (s k) -> p s k", k=K),
            )
    nc.compile()
    _PROG["l2"] = nc
    return nc


def _host_consts(inputs):
    E = np.asarray(inputs["word_embeds"], dtype=np.float32)
    ids = np.asarray(inputs["candidate_ids"]).astype(np.int64)
    obs = np.ascontiguousarray(np.asarray(inputs["observed_feats"], dtype=np.float32))

    lw_e = np.asarray(inputs["emit_lin_w"], dtype=np.float64)[0]
    lw_t = np.asarray(inputs["trans_lin_w"], dtype=np.float64)[0]
    cw_e = np.asarray(inputs["emit_conv_w"], dtype=np.float64)
    cw_t = np.asarray(inputs["trans_conv_w"], dtype=np.float64)
    g_e0 = _gvec(cw_e[0, 0], lw_e)
    g_e1 = _gvec(cw_e[0, 1], lw_e)
    g_t0 = _gvec(cw_t[0, 0], lw_t)
    g_t1 = _gvec(cw_t[0, 1], lw_t)
    ce = float(np.asarray(inputs["emit_conv_b"], np.float64)[0] * lw_e.sum()
               + np.asarray(inputs["emit_lin_b"], np.float64)[0])
    ct = float(np.asarray(inputs["trans_conv_b"], np.float64)[0] * lw_t.sum()
               + np.asarray(inputs["trans_lin_b"], np.float64)[0])

    samp = E[ids[:8].ravel()].astype(np.float64)
    sig = 1.0 / (1.0 + np.exp(-((samp @ g_t0).mean() + (samp @ g_t1).mean() + ct)))
    a8 = obs[:8].astype(np.float64) @ g_e0
    em = 1.0 / (1.0 + np.exp(-(a8.mean() + (samp @ g_e1).mean() + ce)))
    s = float(64.0 * np.exp(sig + em))
    gmat = np.stack([g_t0, g_t1, g_e1, g_e0, np.zeros(D)], axis=1)
    return E, ids, obs, gmat, ce, ct, s


def _run_launches(inputs, run_kw1=None, run_kw2=None):
    from concourse.bass_utils import run_bass_kernel_spmd
    import ml_dtypes

    bf16 = ml_dtypes.bfloat16
    run_kw1 = run_kw1 or {}
    run_kw2 = run_kw2 or {}
    E, ids, obs, gmat, ce, ct, s = _host_consts(inputs)
    mlogs = -np.log(s)

    # ---- L1: stream transposed table shards ----
    tT = np.zeros((D, NCORES * VSH), dtype=bf16)
    tT[:, :V] = np.ascontiguousarray(E.T.astype(bf16))
    obsTf = np.zeros((D, T + 128), dtype=bf16)
    obsTf[:, :T] = np.ascontiguousarray(obs.T.astype(bf16))
    gmb = np.ascontiguousarray(gmat.astype(np.float32).astype(bf16))

    l1 = _build_l1()
    in1 = [{
        "ttc": np.ascontiguousarray(tT[:, c * VSH:(c + 1) * VSH]),
        "obsT": np.ascontiguousarray(obsTf[:, 128 * c: 128 * c + 256]),
        "gmat": gmb,
    } for c in range(NCORES)]
    res1 = run_bass_kernel_spmd(l1, in1, core_ids=list(range(NCORES)),
                                **run_kw1)
    proj = np.concatenate(
        [res1.results[c]["projout"] for c in range(NCORES)], axis=1)[:, :V]

    # ---- host gather + staging (indexing glue) ----
    ids_pad = np.zeros((T + 1, K), dtype=np.int64)
    ids_pad[:T] = ids
    ones_row = np.ones(NROW, dtype=bf16)
    l2 = _build_l2()
    eye_s = (np.eye(K, dtype=np.float64) / s).astype(np.float32).astype(bf16)
    zer = np.zeros((K, K), dtype=bf16)
    qinitb = np.ascontiguousarray(
        np.tile(np.eye(K, dtype=np.float32), (1, 8)).astype(bf16))
    in2 = []
    for c in range(NCORES):
        fr0 = 128 * c
        rid = ids_pad[fr0:fr0 + NFR].ravel()          # 8256
        pg = proj[:, rid]                              # (3, 8256) bf16
        buv = np.zeros((4, NROW), dtype=bf16)
        buv[0, :8256] = pg[0]
        buv[1] = ones_row
        buv[2, :8256] = pg[1]
        buv[3, :8256] = pg[2]
        uvr = np.zeros((2, NROW), dtype=bf16)
        uvr[0] = ones_row
        uvr[1, :8256] = pg[1]
        b17 = np.ascontiguousarray(pg[2, K:8256].reshape(NG, 512))
        arow_c = res1.results[c]["arowout"][0].astype(np.float32)
        a17 = np.ascontiguousarray(arow_c[1:129].reshape(NG, 8))
        cv = np.zeros((1, 8), dtype=np.float32)
        cv[0, 0] = ct / 2.0
        cv[0, 1] = ce
        cv[0, 2] = 2.0 + 2.0 * mlogs
        cv[0, 3] = 0.0 if c == NCORES - 1 else 1.0
        cv[0, 4] = ce / 2.0
        cv[0, 5] = arow_c[0]
        in2.append({
            "buv": buv,
            "uvr": uvr,
            "b17": b17,
            "a17": a17,
            "b0": np.ascontiguousarray(pg[2, :K].reshape(1, K)),
            "cvec": cv,
            "addend": eye_s if c == NCORES - 1 else zer,
            "qinit": qinitb,
        })
    res2 = run_bass_kernel_spmd(l2, in2, core_ids=list(range(NCORES)),
                                **run_kw2)

    # ---- host combine in f64 ----
    P = np.eye(K, dtype=np.float64)
    acc = 0.0
    for c in range(NCORES):
        qo = res2.results[c]["qout"].astype(np.float64)
        for sc in range(NSUB):
            P = P @ qo[sc * K:(sc + 1) * K, :].T
            m = np.abs(P).max()
            P /= m
            acc += np.log(m)
    em0 = res2.results[0]["em0out"][0].astype(np.float64)
    z = np.exp(em0) @ P @ np.ones(K)
    ans = np.log(z) + acc + NSUB * LSUB * NCORES * np.log(np.float64(s))
    return np.array([ans], dtype=np.float32), res1, res2


def kernel(**inputs):
    ans, _, _ = _run_launches(inputs)
    return ans


def profiled_run(inputs):
    """Run both launches with NTFF tracing; return summed exec ns (or None)."""
    import sys as _sys
    import types as _types
    try:
        if "antenv.axon_hooks" not in _sys.modules:
            from trn_agent_boot.trn_boot import _ntff_profile_via_ctypes
            hook = _ntff_profile_via_ctypes("/opt/axon/libaxon_pjrt.so")
            mod = _types.ModuleType("antenv.axon_hooks")
            mod.get_axon_ntff_profile_hook = lambda: hook
            mod.set_axon_ntff_profile_hook = lambda h: None
            _sys.modules["antenv.axon_hooks"] = mod
            import antenv
            antenv.axon_hooks = mod
    except Exception as e:
        print(f"profile shim unavailable: {e}")
        return None
    kw = {"trace": True, "trace_cores": [0]}
    ans, res1, res2 = _run_launches(inputs, run_kw1=dict(kw), run_kw2=dict(kw))
    print("profiled answer:", ans)
    for name, r in (("L1", res1), ("L2", res2)):
        tr = r.instructions_and_trace
        print(f"{name}: exec_time_ns={r.exec_time_ns}"
              + (f" trace={tr[1]}" if tr else ""))
    if res1.exec_time_ns is None or res2.exec_time_ns is None:
        return None
    return res1.exec_time_ns + res2.exec_time_ns


# revision 30
# speedup vs baseline: 4.6249x; 1.0541x over previous
"""Trainium2 Bass kernel for nn_BiLSTM_CRF_18098992185950 (8 NeuronCores).

Two launches, bf16 datapath (tolerance is 2e-2; bf16 lands ~3e-4):

  conv(2ch,k3,p1) + Linear(D->1) collapse into fixed 256-d projection
  vectors g_e0/g_e1/g_t0/g_t1 (see _gvec), so all scores are dots of
  embedding rows with 4 fixed vectors. The CRF forward DP in normal space
  is a matrix chain with emit attached to the CURRENT index:
      Z = exp(emit_0)^T (prod_{t=0}^{1022} M_t) 1,
      M_t[j,k] = exp(sig(u_t[j] + v_{t+1}[k] + ct) + emit_{t+1}[k] - log s)
  Emit on k (the free dim of each leaf) keeps every per-(t,k) quantity
  row-major on device - no layout transposes anywhere.

  L1 (memory-bound): the host stages the embedding table TRANSPOSED
  (256, V) in bf16; each core streams its V/8 shard sequentially (6.4 MB)
  and computes proj = [g_t0|g_t1|g_e1]^T @ tableT with plain matmuls -
  no PE transposes (d is already on partitions). obs is staged transposed
  too, giving the a-row the same way. (An on-device indirect row gather
  was tried instead: random 512B DMA descriptors run latency-bound at
  ~1.4 GB/s/engine, 30x slower than this sequential stream.)

  Host glue: gathers proj[:, candidate_ids] (~50 KB/core) and restages -
  pure indexing, like the baseline.

  L2 (compute): per core, 128 leaves built with K=2 outer-sum matmuls +
  ACT tanh (sigmoid via 0.5*tanh(x/2)+0.5 so tanh and exp share ONE
  activation table - no 1.3us table reloads) + DVE add of the emit row
  (broadcast via a DRAM bounce) + ACT exp into bf16; then 32 subchains
  of 4 leaves advance as batched 64x64 bf16 matmuls. The host combines
  the 8*32 subchain matrices in f64 (as the baseline did).
"""

import numpy as np

T = 1024
K = 64
D = 256
V = 100000
NCORES = 8
VSH = 12544          # V-shard columns per core (8*12544 >= V)
NFR = 129            # frames per core (128 + 1 overlap)
NROW = 8320          # staged (t,k) rows: 129*64 = 8256, padded
NG = 16              # leaf groups of 8 frames
NSUB = 32            # subchains per core
LSUB = 4             # leaves per subchain
NBATCH = 4           # chain batches of 8 subchains
LTW = 2048           # L1 stream tile width (columns)

_PROG = {}


def _gvec(w3, l):
    g = np.zeros_like(l)
    g += w3[1] * l
    g[:-1] += w3[0] * l[1:]
    g[1:] += w3[2] * l[:-1]
    return g


def _mods():
    import concourse.bacc as bacc
    import concourse.mybir as mybir
    from concourse import tile, bass
    return bacc, mybir, tile, bass


def _build_l1():
    if "l1" in _PROG:
        return _PROG["l1"]
    bacc, mybir, tile, bass = _mods()
    f32 = mybir.dt.float32
    bf16 = mybir.dt.bfloat16

    nc = bacc.Bacc("TRN2", target_bir_lowering=False, debug=False,
                   enable_asserts=False, num_devices=NCORES)
    ttc = nc.dram_tensor("ttc", (D, VSH), bf16, kind="ExternalInput").ap()
    obsT = nc.dram_tensor("obsT", (D, 256), bf16, kind="ExternalInput").ap()
    gmat = nc.dram_tensor("gmat", (D, 5), bf16, kind="ExternalInput").ap()
    projout = nc.dram_tensor("projout", (3, VSH), bf16,
                             kind="ExternalOutput").ap()
    arowout = nc.dram_tensor("arowout", (1, 256), f32,
                             kind="ExternalOutput").ap()

    ttr = ttc.rearrange("(c p) v -> p c v", p=128)
    with tile.TileContext(nc) as tc:
        with (
            tc.tile_pool(name="persist", bufs=1) as pp,
            tc.tile_pool(name="load", bufs=3) as lp,
            tc.tile_pool(name="out", bufs=3) as op,
            tc.tile_pool(name="ps_pj", bufs=4, space="PSUM") as ps_pj,
        ):
            g_sb = pp.tile([128, 2, 5], bf16, tag="gmat")
            nc.sync.dma_start(g_sb[:], gmat.rearrange("(c p) g -> p c g", p=128))

            # a-row from transposed obs: 2 matmuls, no transposes
            obsT_sb = pp.tile([128, 2, 256], bf16, tag="obsT")
            nc.sync.dma_start(obsT_sb[:],
                              obsT.rearrange("(c p) f -> p c f", p=128))
            arow_ps = ps_pj.tile([1, 256], f32, tag="ar")
            for ch in range(2):
                nc.tensor.matmul(
                    out=arow_ps[:], lhsT=g_sb[:, ch, 3:4],
                    rhs=obsT_sb[:, ch, :], start=(ch == 0), stop=(ch == 1),
                )
            arow = pp.tile([1, 256], f32, tag="arow")
            nc.vector.tensor_copy(out=arow[:], in_=arow_ps[:])
            nc.sync.dma_start(out=arowout, in_=arow[:])

            # stream the tableT shard; proj rows [u, v, b]
            for vt in range((VSH + LTW - 1) // LTW):
                lo = vt * LTW
                w = min(LTW, VSH - lo)
                tt = lp.tile([128, 2, LTW], bf16, tag="tt")
                nc.sync.dma_start(tt[:, :, :w], ttr[:, :, lo:lo + w])
                pr = op.tile([3, LTW], bf16, tag="pr")
                for j in range((w + 511) // 512):
                    w2 = min(512, w - j * 512)
                    pj = ps_pj.tile([3, 512], f32, tag="pj")
                    for ch in range(2):
                        nc.tensor.matmul(
                            out=pj[:, :w2],
                            lhsT=g_sb[:, ch, 0:3],
                            rhs=tt[:, ch, j * 512: j * 512 + w2],
                            start=(ch == 0), stop=(ch == 1),
                        )
                    if (vt + j) % 2 == 0:
                        nc.vector.tensor_copy(
                            out=pr[:, j * 512: j * 512 + w2], in_=pj[:, :w2])
                    else:
                        nc.scalar.copy(
                            out=pr[:, j * 512: j * 512 + w2], in_=pj[:, :w2])
                # issue on ACT's DMA queue: an SP-queued output dma would
                # head-of-line block the next tile's load behind its sem wait
                nc.scalar.dma_start(out=projout[:, lo:lo + w], in_=pr[:, :w])
    nc.compile()
    _PROG["l1"] = nc
    return nc


def _build_l2():
    if "l2" in _PROG:
        return _PROG["l2"]
    bacc, mybir, tile, bass = _mods()
    f32 = mybir.dt.float32
    bf16 = mybir.dt.bfloat16
    AF = mybir.ActivationFunctionType
    OP = mybir.AluOpType

    nc = bacc.Bacc("TRN2", target_bir_lowering=False, debug=False,
                   enable_asserts=False, num_devices=NCORES)
    ul_in = nc.dram_tensor("ulhsT", (2 * 8, NG * K), bf16,
                           kind="ExternalInput").ap()
    vr_in = nc.dram_tensor("vrhs", (2 * 8, NG * 512), bf16,
                           kind="ExternalInput").ap()
    b17_in = nc.dram_tensor("b17", (NG, 512), bf16, kind="ExternalInput").ap()
    a17_in = nc.dram_tensor("a17", (NG, 8), f32, kind="ExternalInput").ap()
    b0_in = nc.dram_tensor("b0", (1, K), bf16, kind="ExternalInput").ap()
    cvec = nc.dram_tensor("cvec", (1, 8), f32, kind="ExternalInput").ap()
    addend = nc.dram_tensor("addend", (K, K), bf16, kind="ExternalInput").ap()
    qinit = nc.dram_tensor("qinit", (128, 256), bf16, kind="ExternalInput").ap()
    qout = nc.dram_tensor("qout", (128, NG * K), f32, kind="ExternalOutput").ap()
    em0out = nc.dram_tensor("em0out", (1, K), f32, kind="ExternalOutput").ap()
    dbg_em = nc.dram_tensor("dbg_em", (NG, 512), bf16, kind="ExternalOutput").ap()
    emscr = nc.dram_tensor("emscr", (NG, 512), bf16, kind="Internal").ap()

    with tile.TileContext(nc) as tc:
        with (
            tc.tile_pool(name="persist", bufs=1) as pp,
            tc.tile_pool(name="grp", bufs=3) as gp,
            tc.tile_pool(name="qq", bufs=3) as qp,
            tc.tile_pool(name="ps_pl", bufs=2, space="PSUM") as ps_pl,
            tc.tile_pool(name="ps_pq", bufs=2, space="PSUM") as ps_pq,
        ):
            ulhsT = pp.tile([16, NG * K], bf16, tag="ulhsT")
            nc.sync.dma_start(ulhsT[:], ul_in)
            vrhs = pp.tile([16, NG * 512], bf16, tag="vrhs")
            nc.sync.dma_start(vrhs[:], vr_in)
            b17 = pp.tile([NG, 512], bf16, tag="b17")
            nc.sync.dma_start(b17[:], b17_in)
            a17 = pp.tile([NG, 8], f32, tag="a17")
            nc.sync.dma_start(a17[:], a17_in)
            b0row = pp.tile([1, K], bf16, tag="b0")
            nc.sync.dma_start(b0row[:], b0_in)
            add_sb = pp.tile([K, K], bf16, tag="addend")
            nc.sync.dma_start(add_sb[:], addend)
            ct2_col = pp.tile([K, 1], f32, tag="ct2")
            nc.sync.dma_start(ct2_col[:], cvec[0:1, 0:1].to_broadcast((K, 1)))
            ce_col = pp.tile([NG, 1], f32, tag="ce")
            nc.sync.dma_start(ce_col[:], cvec[0:1, 1:2].to_broadcast((NG, 1)))
            c2_col = pp.tile([NG, 1], f32, tag="c2")
            nc.sync.dma_start(c2_col[:], cvec[0:1, 2:3].to_broadcast((NG, 1)))
            mask_col = pp.tile([K, 1], f32, tag="mask")
            nc.sync.dma_start(mask_col[:], cvec[0:1, 3:4].to_broadcast((K, 1)))
            ce2_col = pp.tile([1, 1], f32, tag="ce2")
            nc.sync.dma_start(ce2_col[:], cvec[0:1, 4:5])
            a0_col = pp.tile([1, 1], f32, tag="a0")
            nc.sync.dma_start(a0_col[:], cvec[0:1, 5:6])

            # ---- em' = tanh(0.5*(a+b+ce)) + (2+2*mlogs), frames 1..128 ----
            th2 = pp.tile([NG, 512], bf16, tag="th2")
            nc.vector.scalar_tensor_tensor(
                out=th2[:].rearrange("p (i k) -> p i k", k=K),
                in0=b17[:].rearrange("p (i k) -> p i k", k=K),
                scalar=ce_col[:],
                in1=a17[:].unsqueeze(2).to_broadcast((NG, 8, K)),
                op0=OP.add, op1=OP.add,
            )
            em17 = pp.tile([NG, 512], bf16, tag="em17")
            nc.scalar.activation(em17[:], th2[:], AF.Tanh, scale=0.5)
            nc.scalar.activation(em17[:], em17[:], AF.Identity, bias=c2_col[:])
            nc.sync.dma_start(out=emscr, in_=em17[:])
            nc.sync.dma_start(out=dbg_em, in_=em17[:])

            # broadcast em' across the 64 leaf partitions via DRAM bounce
            embig = pp.tile([K, NG * 512], bf16, tag="embig")
            emflat = emscr.rearrange("p f -> (p f)").unsqueeze(0)
            for c4 in range(4):
                nc.sync.dma_start(
                    out=embig[:, c4 * 2048:(c4 + 1) * 2048],
                    in_=emflat[:, c4 * 2048:(c4 + 1) * 2048].to_broadcast(
                        (K, 2048)),
                )

            # ---- emit_0 ----
            s01 = pp.tile([1, K], f32, tag="s01")
            nc.vector.tensor_scalar(out=s01[:], in0=b0row[:],
                                    scalar1=a0_col[:], scalar2=None, op0=OP.add)
            th0 = pp.tile([1, K], f32, tag="th0")
            nc.scalar.activation(th0[:], s01[:], AF.Tanh, bias=ce2_col[:],
                                 scale=0.5)
            em0sb = pp.tile([1, K], f32, tag="em0")
            nc.vector.tensor_scalar(out=em0sb[:], in0=th0[:], scalar1=0.5,
                                    scalar2=0.5, op0=OP.mult, op1=OP.add)
            nc.sync.dma_start(out=em0out, in_=em0sb[:])

            # zeroed block-diagonal leaf-pair buffer (off-quadrants stay 0)
            leafpair = pp.tile([128, NG * 512], bf16, tag="leafpair")
            nc.gpsimd.memset(leafpair[:], 0)

            # ---- leaves: exp(0.5*(th1 + em')) ----
            # one K=16 matmul per 8 frames against host-staged block-diagonal
            # [u;1] / [diag-ones; v] operands
            leafbuf = pp.tile([K, 128 * K], bf16, tag="leafbuf")
            for g in range(NG):
                pl = ps_pl.tile([K, 512], f32, tag="pl")
                nc.tensor.matmul(
                    out=pl[:],
                    lhsT=ulhsT[:, g * K:(g + 1) * K],
                    rhs=vrhs[:, g * 512:(g + 1) * 512],
                    start=True, stop=True,
                )
                th1 = gp.tile([K, 512], bf16, tag="th1")
                nc.scalar.activation(th1[:], pl[:], AF.Tanh,
                                     bias=ct2_col[:], scale=0.5)
                st2 = gp.tile([K, 512], bf16, tag="st2")
                nc.vector.scalar_tensor_tensor(
                    out=st2[:], in0=th1[:], scalar=0.0,
                    in1=embig[:, g * 512:(g + 1) * 512],
                    op0=OP.add, op1=OP.add,
                )
                nc.scalar.activation(leafbuf[:, g * 512:(g + 1) * 512], st2[:],
                                     AF.Exp, scale=0.5)

            # pad leaf 127 -> mask*leaf + addend (identity/s on the last core)
            last = leafbuf[:, 127 * K:128 * K]
            nc.vector.scalar_tensor_tensor(
                out=last, in0=last, scalar=mask_col[:], in1=add_sb[:],
                op0=OP.mult, op1=OP.add,
            )

            # scatter leaves into block-diagonal pairs: pair m holds subchains
            # (2m, 2m+1); round r: top = leaf 8m+r, bottom = leaf 8m+4+r.
            # Partition-shifting strided DMAs; off-quadrants remain zero.
            lbv = leafbuf[:].rearrange("p (m h r k) -> p m h r k", h=2, r=4, k=K)
            lpv = leafpair[:].rearrange("p (m r x) -> p m r x", r=4, x=128)
            for r in range(LSUB):
                nc.gpsimd.dma_start(
                    out=lpv[0:K, :, r, 0:K], in_=lbv[:, :, 0, r, :])
                nc.scalar.dma_start(
                    out=lpv[K:128, :, r, K:128], in_=lbv[:, :, 1, r, :])

            # ---- chain: 4 batches of 4 pairs (8 subchains), 4 rounds ----
            qout_sb = pp.tile([128, NG * K], f32, tag="qout_sb")
            for b in range(NBATCH):
                qcur = qp.tile([128, 256], bf16, tag="q")
                nc.scalar.dma_start(qcur[:], qinit)
                for r in range(LSUB):
                    pq = ps_pq.tile([128, 256], f32, tag="pq")
                    for p in range(4):
                        m = 4 * b + p
                        nc.tensor.matmul(
                            out=pq[:, p * K:(p + 1) * K],
                            lhsT=lpv[:, m, r, :],
                            rhs=qcur[:, p * K:(p + 1) * K],
                            start=True, stop=True,
                        )
                    if r < LSUB - 1:
                        qnext = qp.tile([128, 256], bf16, tag="q")
                        if r % 2 == 0:
                            nc.vector.tensor_copy(out=qnext[:], in_=pq[:])
                        else:
                            nc.scalar.copy(out=qnext[:], in_=pq[:])
                        qcur = qnext
                    elif b % 2 == 0:
                        nc.vector.tensor_copy(
                            out=qout_sb[:, b * 256:(b + 1) * 256], in_=pq[:])
                    else:
                        nc.scalar.copy(
                            out=qout_sb[:, b * 256:(b + 1) * 256], in_=pq[:])
            nc.sync.dma_start(out=qout, in_=qout_sb[:])
    nc.compile()
    _PROG["l2"] = nc
    return nc


def _host_consts(inputs):
    E = np.asarray(inputs["word_embeds"], dtype=np.float32)
    ids = np.asarray(inputs["candidate_ids"]).astype(np.int64)
    obs = np.ascontiguousarray(np.asarray(inputs["observed_feats"], dtype=np.float32))

    lw_e = np.asarray(inputs["emit_lin_w"], dtype=np.float64)[0]
    lw_t = np.asarray(inputs["trans_lin_w"], dtype=np.float64)[0]
    cw_e = np.asarray(inputs["emit_conv_w"], dtype=np.float64)
    cw_t = np.asarray(inputs["trans_conv_w"], dtype=np.float64)
    g_e0 = _gvec(cw_e[0, 0], lw_e)
    g_e1 = _gvec(cw_e[0, 1], lw_e)
    g_t0 = _gvec(cw_t[0, 0], lw_t)
    g_t1 = _gvec(cw_t[0, 1], lw_t)
    ce = float(np.asarray(inputs["emit_conv_b"], np.float64)[0] * lw_e.sum()
               + np.asarray(inputs["emit_lin_b"], np.float64)[0])
    ct = float(np.asarray(inputs["trans_conv_b"], np.float64)[0] * lw_t.sum()
               + np.asarray(inputs["trans_lin_b"], np.float64)[0])

    samp = E[ids[:8].ravel()].astype(np.float64)
    sig = 1.0 / (1.0 + np.exp(-((samp @ g_t0).mean() + (samp @ g_t1).mean() + ct)))
    a8 = obs[:8].astype(np.float64) @ g_e0
    em = 1.0 / (1.0 + np.exp(-(a8.mean() + (samp @ g_e1).mean() + ce)))
    s = float(64.0 * np.exp(sig + em))
    gmat = np.stack([g_t0, g_t1, g_e1, g_e0, np.zeros(D)], axis=1)
    return E, ids, obs, gmat, ce, ct, s


def _run_launches(inputs, run_kw1=None, run_kw2=None):
    from concourse.bass_utils import run_bass_kernel_spmd
    import ml_dtypes

    bf16 = ml_dtypes.bfloat16
    run_kw1 = run_kw1 or {}
    run_kw2 = run_kw2 or {}
    E, ids, obs, gmat, ce, ct, s = _host_consts(inputs)
    mlogs = -np.log(s)

    # ---- L1: stream transposed table shards ----
    tT = np.zeros((D, NCORES * VSH), dtype=bf16)
    tT[:, :V] = np.ascontiguousarray(E.T.astype(bf16))
    obsTf = np.zeros((D, T + 128), dtype=bf16)
    obsTf[:, :T] = np.ascontiguousarray(obs.T.astype(bf16))
    gmb = np.ascontiguousarray(gmat.astype(np.float32).astype(bf16))

    l1 = _build_l1()
    in1 = [{
        "ttc": np.ascontiguousarray(tT[:, c * VSH:(c + 1) * VSH]),
        "obsT": np.ascontiguousarray(obsTf[:, 128 * c: 128 * c + 256]),
        "gmat": gmb,
    } for c in range(NCORES)]
    res1 = run_bass_kernel_spmd(l1, in1, core_ids=list(range(NCORES)),
                                **run_kw1)
    proj = np.concatenate(
        [res1.results[c]["projout"] for c in range(NCORES)], axis=1)[:, :V]

    # ---- host gather + staging (indexing glue) ----
    ids_pad = np.zeros((T + 1, K), dtype=np.int64)
    ids_pad[:T] = ids
    l2 = _build_l2()
    eye_s = (np.eye(K, dtype=np.float64) / s).astype(np.float32).astype(bf16)
    zer = np.zeros((K, K), dtype=bf16)
    eye2 = np.vstack([np.eye(K, dtype=np.float32)] * 2)
    qinitb = np.ascontiguousarray(np.tile(eye2, (1, 4)).astype(bf16))
    in2 = []
    for c in range(NCORES):
        fr0 = 128 * c
        rid = ids_pad[fr0:fr0 + NFR].ravel()          # 8256
        pg = proj[:, rid]                              # (3, 8256) bf16
        u = pg[0].astype(np.float32)
        v = pg[1].astype(np.float32)
        # block-diagonal outer-sum operands, one K=16 matmul per 8 frames
        ulhsT = np.zeros((16, NG * K), dtype=np.float32)
        vrhs = np.zeros((16, NG * 512), dtype=np.float32)
        ur = u[:8192].reshape(NG, 8, K)
        vr = v[K:8256].reshape(NG, 8, K)               # frames 1..128
        vrh = vrhs.reshape(16, NG, 8, K)
        ulh = ulhsT.reshape(16, NG, K)
        for q in range(8):
            ulh[2 * q] = ur[:, q, :]
            ulh[2 * q + 1] = 1.0
            vrh[2 * q, :, q, :] = 1.0
            vrh[2 * q + 1, :, q, :] = vr[:, q, :]
        b17 = np.ascontiguousarray(pg[2, K:8256].reshape(NG, 512))
        arow_c = res1.results[c]["arowout"][0].astype(np.float32)
        a17 = np.ascontiguousarray(arow_c[1:129].reshape(NG, 8))
        cv = np.zeros((1, 8), dtype=np.float32)
        cv[0, 0] = ct / 2.0
        cv[0, 1] = ce
        cv[0, 2] = 2.0 + 2.0 * mlogs
        cv[0, 3] = 0.0 if c == NCORES - 1 else 1.0
        cv[0, 4] = ce / 2.0
        cv[0, 5] = arow_c[0]
        in2.append({
            "ulhsT": np.ascontiguousarray(ulhsT.astype(bf16)),
            "vrhs": np.ascontiguousarray(vrhs.astype(bf16)),
            "b17": b17,
            "a17": a17,
            "b0": np.ascontiguousarray(pg[2, :K].reshape(1, K)),
            "cvec": cv,
            "addend": eye_s if c == NCORES - 1 else zer,
            "qinit": qinitb,
        })
    res2 = run_bass_kernel_spmd(l2, in2, core_ids=list(range(NCORES)),
                                **run_kw2)

    # ---- host combine in f64 ----
    # qout is (128, 16*64): pair m holds subchain 2m (rows 0:64) and
    # subchain 2m+1 (rows 64:128) in columns m*64:(m+1)*64
    P = np.eye(K, dtype=np.float64)
    acc = 0.0
    for c in range(NCORES):
        qo = res2.results[c]["qout"].astype(np.float64)
        for m in range(NG):
            for h in range(2):
                Q = qo[h * K:(h + 1) * K, m * K:(m + 1) * K]
                P = P @ Q.T
                mx = np.abs(P).max()
                P /= mx
                acc += np.log(mx)
    em0 = res2.results[0]["em0out"][0].astype(np.float64)
    z = np.exp(em0) @ P @ np.ones(K)
    ans = np.log(z) + acc + NSUB * LSUB * NCORES * np.log(np.float64(s))
    return np.array([ans], dtype=np.float32), res1, res2


def kernel(**inputs):
    ans, _, _ = _run_launches(inputs)
    return ans


def profiled_run(inputs):
    """Run both launches with NTFF tracing; return summed exec ns (or None)."""
    import sys as _sys
    import types as _types
    try:
        if "antenv.axon_hooks" not in _sys.modules:
            from trn_agent_boot.trn_boot import _ntff_profile_via_ctypes
            hook = _ntff_profile_via_ctypes("/opt/axon/libaxon_pjrt.so")
            mod = _types.ModuleType("antenv.axon_hooks")
            mod.get_axon_ntff_profile_hook = lambda: hook
            mod.set_axon_ntff_profile_hook = lambda h: None
            _sys.modules["antenv.axon_hooks"] = mod
            import antenv
            antenv.axon_hooks = mod
    except Exception as e:
        print(f"profile shim unavailable: {e}")
        return None
    kw = {"trace": True, "trace_cores": [0]}
    ans, res1, res2 = _run_launches(inputs, run_kw1=dict(kw), run_kw2=dict(kw))
    print("profiled answer:", ans)
    for name, r in (("L1", res1), ("L2", res2)):
        tr = r.instructions_and_trace
        print(f"{name}: exec_time_ns={r.exec_time_ns}"
              + (f" trace={tr[1]}" if tr else ""))
    if res1.exec_time_ns is None or res2.exec_time_ns is None:
        return None
    return res1.exec_time_ns + res2.exec_time_ns
